# revision 43
# baseline (speedup 1.0000x reference)
"""Causal self-attention (B=4, T=2048, C=1024, H=16) on 8 TRN2 NeuronCores.

Tensor-parallel over heads: each core owns 2 heads (q/k/v column slice of
c_attn, matching row slice of c_proj) and computes a partial projection
output over the full batch; partials are summed on the host.

Device layout notes (per core):
  - x is staged transposed+bf16 on the host: xt [C, B*T].
  - QKV matmuls produce qT/kT [128, T] per batch (2 heads stacked 64+64 on
    partitions, head_dim on partitions) and a vT staging tile that is
    PE-transposed into token-major v_aug tiles carrying an extra ones
    column (yields the softmax denominator for free during the AV matmul).
  - Scores are computed transposed, S^T [k_tok, q_tok], exp(s/8) runs on
    ACT straight from PSUM into SBUF (no max-subtraction: scores are O(1)
    here, exp is overflow-safe). Causality = trimmed matmul/exp ranges +
    one static triangular mask on diagonal 128-blocks. For a diagonal
    k-tile with local offset `off`, only q columns [off:512) are computed:
    head0 scores land at sc[:, off:512], head1 at sc[:, 512:1024-off], so
    a single contiguous exp call covers both.
  - AV accumulates Y^T in PSUM with M=128 stationaries: head0 via
    [v(64) | 1 | 0(63)] (y rows 0..64, denominator row 64), head1 via
    [0(32) | 1 | 0(31) | v(64)] (y rows 64..128, denominator row 32).
  - Normalization is DMA-free: the two denominator rows are staged in a
    zeroed [65, 512] bf16 tile at their native partitions (64 and 32), a
    single K=65 selector matmul gathers and broadcasts them across all
    128 partitions into PSUM (it depends only on DVE copies, so it never
    stalls the PE on ACT), 1/den = exp(-ln(den)) runs on ACT over the
    broadcast tile (ln and exp share the natural_log_exp_and_others
    table set, so no table switch), and two tensor_muls apply it.
  - tri/ident mask constants ship from the host as tiny extra inputs.
Output is the transposed partial projection outp [C, B*T] f32.
"""

import numpy as np
import ml_dtypes

import concourse.bass as bass
import concourse.tile as tile
from concourse import mybir
from concourse.bass_utils import run_bass_kernel_spmd
from concourse.vector_clock import ScopedClock, VectorClock

BF16 = mybir.dt.bfloat16
F32 = mybir.dt.float32
EXPF = mybir.ActivationFunctionType.Exp
LNF = mybir.ActivationFunctionType.Ln

B, T, C, H = 4, 2048, 1024, 16
D = C // H          # 64
NCORES = 8
HPC = H // NCORES   # 2 heads per core
DC = HPC * D        # 128 channels per core
SCALE = 1.0 / float(np.sqrt(D))
ELIDE_INCS = False


def _patch_tile_drain():
    """walrus's Drain template rejects >2 sync waits; split the tail-drain
    waits one proc per drain."""
    if getattr(tile.TileContext, "_drain_patched", False):
        return

    def _drain_and_barrier(self, tick_clock, wait_clock):
        nc = self.nc
        gc = tick_clock.global_clock
        n = len(gc)
        for p in range(n):
            if gc[p] > 0:
                d = nc.sync.drain()
                vc = VectorClock([gc[p] if i == p else 0 for i in range(n)])
                wait_clock.add_sem_waits(d.ins, ScopedClock({None: vc}))
        nc.sync.drain()
        nc.all_engine_barrier()
        assert self.sems is not None
        popped = nc._tile_sem_poison_stack.pop()
        assert popped is self._sem_poison
        nc.clear_and_free_semaphores(list(self.sems.allocated().values()))
        nc.all_engine_barrier()

    tile.TileContext._drain_and_barrier = _drain_and_barrier
    tile.TileContext._drain_patched = True


def _elide_unwaited_incs(nc, sem_ids=(172,)):
    """Drop per-instruction completion increments whose cumulative count
    no wait ever references, remapping the surviving wait thresholds.

    Every PE matmul carries a then-inc on its engine semaphore (~26ns of
    serialized EVT_SEM traffic each), but only ~1/3 of the counts are
    actually waited on. Semantics are preserved exactly: a wait with
    threshold v fired at the inc whose cumulative count was v; that inc
    is kept, and v is remapped to the inc's rank among kept incs.
    """
    import bass_rust

    all_insts = [
        ins for f in nc.m.functions for bb in f.blocks
        for ins in bb.instructions
    ]
    for sem in sem_ids:
        waited = set()
        for ins in all_insts:
            si = ins.sync_info
            if si is None:
                continue
            for w in (si.on_wait or []):
                if w.id == sem and str(w.wait_mode) == "sem-ge-imm":
                    waited.add(w.wait_value)
                elif w.id == sem:
                    # unknown wait mode on this sem: bail out entirely
                    return
        # pass 1: walk incs in program order, decide keep/drop, build the
        # old-count -> new-count remap
        cum = 0
        kept_cums = []
        keeps = {}  # id(ins) -> list of keep-decisions for this sem
        for ins in all_insts:
            si = ins.sync_info
            if si is None or not si.on_update:
                continue
            for u in si.on_update:
                if u.id != sem:
                    continue
                if str(u.update_mode) != "sem-inc" or u.update_value != 1:
                    return  # non-standard update: bail out
                cum += 1
                keep = cum in waited
                if keep:
                    kept_cums.append(cum)
                keeps.setdefault(id(ins), []).append(keep)
        if not kept_cums or max(waited, default=0) > cum:
            return
        import bisect

        def remap(v):
            return bisect.bisect_right(kept_cums, v)

        # pass 2: rewrite sync_infos
        for ins in all_insts:
            si = ins.sync_info
            if si is None:
                continue
            decisions = keeps.get(id(ins), [])
            di = 0
            new_upd = []
            changed = False
            for u in (si.on_update or []):
                if u.id == sem:
                    keep = decisions[di]
                    di += 1
                    if not keep:
                        changed = True
                        continue
                new_upd.append(u)
            new_wait = []
            for w in (si.on_wait or []):
                if w.id == sem:
                    nv = remap(w.wait_value)
                    if nv != w.wait_value:
                        changed = True
                        w = bass_rust.SyncWait(
                            sync_type=w.sync_type, id=w.id,
                            ant_name=w.ant_name, wait_mode=w.wait_mode,
                            wait_value=nv, wait_reg=w.wait_reg,
                        )
                new_wait.append(w)
            if changed:
                ins.sync_info = bass_rust.SyncInfo(
                    on_wait=new_wait, on_update=new_upd
                )


def _split_excess_waits(nc, max_waits=1):
    """walrus's per-instruction sync-wait slot limit is 2; hoist excess
    waits onto same-engine nops inserted just before the instruction."""
    import bass_rust

    snapshots = [
        (bb, list(bb.instructions)) for f in nc.m.functions for bb in f.blocks
    ]
    for bb, insts in snapshots:
        new_list = []
        changed = False
        for inst in insts:
            si = inst.sync_info
            waits = list(si.on_wait) if (si and si.on_wait) else []
            if len(waits) > max_waits:
                changed = True
                excess, keep = waits[:-max_waits], waits[-max_waits:]
                eng = nc.engines[inst.engine]
                for i in range(0, len(excess), max_waits):
                    nop_inst = eng.nop().ins
                    nop_inst.sync_info = bass_rust.SyncInfo(
                        on_wait=list(excess[i:i + max_waits]), on_update=[]
                    )
                    new_list.append(nop_inst)
                inst.sync_info = bass_rust.SyncInfo(
                    on_wait=keep,
                    on_update=list(si.on_update) if si.on_update else [],
                )
            new_list.append(inst)
        bb.instructions = new_list


def build_attention_nc(nb: int = B, tb: int = T):
    """One-core program; SPMD across cores via per-core input values."""
    assert tb % 512 == 0
    ntok = nb * tb
    nqc = tb // 512          # q chunks per batch
    nkt = tb // 128          # k tiles per batch
    ncc = C // 128           # contraction chunks for QKV

    nc = bass.Bass("TRN2", target_bir_lowering=False, debug=False)
    xt = nc.dram_tensor("xt", [C, ntok], BF16, kind="ExternalInput").ap()
    wqkv = nc.dram_tensor("wqkv", [C, 3 * DC], BF16, kind="ExternalInput").ap()
    wp = nc.dram_tensor("wp", [DC, C], BF16, kind="ExternalInput").ap()
    trid = nc.dram_tensor("tri", [128, 128], BF16, kind="ExternalInput").ap()
    identd = nc.dram_tensor("ident", [128, 128], BF16, kind="ExternalInput").ap()
    bseld = nc.dram_tensor("bsel", [128, 128], BF16, kind="ExternalInput").ap()
    outp = nc.dram_tensor("outp", [C, ntok], BF16, kind="ExternalOutput").ap()

    with tile.TileContext(nc) as tc:
        with (
            tc.tile_pool(name="const", bufs=1) as const,
            tc.tile_pool(name="xtp", bufs=8 * ncc) as xtp,
            tc.tile_pool(name="qkp", bufs=2) as qkp,
            tc.tile_pool(name="vap", bufs=2) as vap,
            tc.tile_pool(name="ep", bufs=6) as ep,
            tc.tile_pool(name="nstg", bufs=4) as nstg,
            tc.tile_pool(name="ynp", bufs=4) as ynp,
            tc.tile_pool(name="ostg", bufs=4) as ostg,
            tc.tile_pool(name="mmps", bufs=2, space="PSUM") as mmps,
            tc.tile_pool(name="scps", bufs=2, space="PSUM") as scps,
            tc.tile_pool(name="y0ps", bufs=1, space="PSUM") as y0ps,
            tc.tile_pool(name="y1ps", bufs=1, space="PSUM") as y1ps,
        ):
            # ---- constants ----
            # wqkv via direct DMA on gpsimd (fast serial stream, needed
            # first); batch-0 x tiles go through the 16 DMA rings in
            # parallel (emit_xt below), so QKV t4=0 can start ~4us in.
            # wqkv rides the scalar hwdge queue: it starts ~2.8us into the
            # kernel while gpsimd's software-DGE path only comes up ~8us in.
            # wqkv chunks are interleaved between the first batch-0 x
            # tiles on the sync+scalar hwdge queues (emitted inside
            # emit_xt below): chunk c rides after x-tile c//2 on queue
            # c%2, so the QKV chain's LDWEIGHTS and x arrivals pace
            # together instead of one stream starving the other.
            wqkv_sb = const.tile([128, ncc * 3 * DC], BF16)
            # wp/ident/tri ride the scalar engine's direct-DMA path (idle
            # until the first exp) so they don't delay gpsimd's x loads;
            # their DMAs are emitted after batch-0's x loads (below).
            wp_sb = const.tile([128, C], BF16)
            ident = const.tile([128, 128], BF16)
            tri = const.tile([128, 128], BF16)
            # denominator gather/broadcast selectors: row 64 maps den0 to
            # output partitions 0-63, row 32 maps den1 to 64-127.
            bsel = const.tile([128, 128], BF16)

            def emit_late_consts():
                nc.gpsimd.dma_start(out=wp_sb, in_=wp)
                nc.gpsimd.dma_start(out=ident, in_=identd)
                nc.gpsimd.dma_start(out=tri, in_=trid)
                nc.gpsimd.dma_start(out=bsel, in_=bseld)

            # ---------- emission helpers (software-pipelined schedule) ----
            def emit_xt(b, spread=False):
                # gpsimd-direct DMAs: no serial SWDGE issue cost on the sync
                # sequencer. For batch 0 (spread=True) fan the first token
                # chunk out across idle engines' direct-DMA paths so QKV
                # work can start ~3us in instead of waiting on one serial
                # stream.
                t0 = b * tb
                xts = {}
                for t4 in range(tb // 512):
                    for ci in range(ncc):
                        xtile = xtp.tile([128, 512], BF16, name="xtile")
                        if spread:
                            eng = (nc.sync, nc.scalar, nc.gpsimd,
                                   nc.gpsimd)[t4]
                        else:
                            eng = nc.gpsimd
                        eng.dma_start(
                            out=xtile,
                            in_=xt[128 * ci:128 * (ci + 1),
                                   t0 + 512 * t4:t0 + 512 * (t4 + 1)],
                        )
                        if spread and t4 < 2 and ci < 4:
                            wi = 2 * ci + t4
                            eng.dma_start(
                                out=wqkv_sb[:, wi * 3 * DC:
                                            (wi + 1) * 3 * DC],
                                in_=wqkv[128 * wi:128 * (wi + 1), :],
                            )
                        xts[(ci, t4)] = xtile
                return xts

            def alloc_qkv():
                return (
                    qkp.tile([128, tb], BF16, tag="qT", name="qT"),
                    qkp.tile([128, tb], BF16, tag="kT", name="kT"),
                    qkp.tile([128, tb], BF16, tag="vTs", name="vTs"),
                )

            def _qkv_span(xts, tiles, g, ps, lo, hi):
                # t4-major group order so batch-0 QKV starts after only the
                # first token-chunk of x has landed.
                t4, oi = divmod(g, 3)
                for ci in range(lo, hi):
                    nc.tensor.matmul(
                        ps[0],
                        lhsT=wqkv_sb[
                            :, ci * 3 * DC + oi * DC:ci * 3 * DC + (oi + 1) * DC
                        ],
                        rhs=xts[(ci, t4)],
                        start=(ci == 0),
                        stop=(ci == ncc - 1),
                    )
                if hi == ncc:
                    # high priority: this CAST recycles the QKV PSUM bank
                    # that gates the group after next; stuck behind ost
                    # CASTs it stalls qkv-start matmuls at the PE head.
                    with tc.high_priority(offset=1 << 20):
                        nc.vector.tensor_copy(
                            tiles[oi][:, 512 * t4:512 * (t4 + 1)], ps[0]
                        )

            def qkv_group_steps(xts, tiles, g):
                ps = [None]

                def s1():
                    ps[0] = mmps.tile([128, 512], F32, tag="mm", name="ps")
                    _qkv_span(xts, tiles, g, ps, 0, 3)

                return [
                    s1,
                    lambda: _qkv_span(xts, tiles, g, ps, 3, 6),
                    lambda: _qkv_span(xts, tiles, g, ps, 6, ncc),
                ]

            def emit_qkv_group(xts, tiles, g):
                for s in qkv_group_steps(xts, tiles, g):
                    s()

            def _vaug_init(va0, va1):
                va0v = va0.rearrange("p (t c) -> p t c", c=128)
                va1v = va1.rearrange("p (t c) -> p t c", c=128)
                nc.vector.memset(va0v[:, :, 64:128], 0.0)
                nc.vector.memset(va0v[:, :, 64:65], 1.0)
                nc.vector.memset(va1v[:, :, 0:64], 0.0)
                nc.vector.memset(va1v[:, :, 32:33], 1.0)

            def _vaug_grp(vTs, va0, va1, g):
                va0v = va0.rearrange("p (t c) -> p t c", c=128)
                va1v = va1.rearrange("p (t c) -> p t c", c=128)
                tps = mmps.tile([128, 512], BF16, tag="mm", name="tps")
                for j in range(4):
                    nc.tensor.transpose(
                        tps[:, 128 * j:128 * (j + 1)],
                        vTs[:, 128 * (4 * g + j):128 * (4 * g + j + 1)],
                        ident,
                    )
                tpsv = tps.rearrange("p (t c) -> p t c", c=128)
                nc.vector.tensor_copy(
                    va0v[:, 4 * g:4 * g + 4, 0:64], tpsv[:, 0:4, 0:64]
                )
                nc.vector.tensor_copy(
                    va1v[:, 4 * g:4 * g + 4, 64:128], tpsv[:, 0:4, 64:128]
                )

            def vaug_steps(vTs, out_holder):
                # head0: per ktile 128 cols = [v(64) | 1 | 0(63)]
                # head1: per ktile 128 cols = [0(32) | 1 | 0(31) | v(64)]
                va0 = vap.tile([128, nkt * 128], BF16, tag="va0", name="va0")
                va1 = vap.tile([128, nkt * 128], BF16, tag="va1", name="va1")
                out_holder["va"] = (va0, va1)
                return [lambda: _vaug_init(va0, va1)] + [
                    lambda g=g: _vaug_grp(vTs, va0, va1, g)
                    for g in range(nkt // 4)
                ]

            def emit_vaug(vTs):
                holders = {}
                for s in vaug_steps(vTs, holders):
                    s()
                return holders["va"]

            def emit_attention_qc(qc, qT, kT, va0, va1, fill,
                                  reserve=False):
                q0 = 512 * qc
                y0 = y0ps.tile([128, 512], F32)
                y1 = y1ps.tile([128, 512], F32)
                nkts = 4 * (qc + 1)
                def scores_exp(kt):
                    # causally trimmed scores: head0 -> sc[:, off:512],
                    # head1 -> sc[:, 512:1024-off] so one contiguous exp
                    # covers both heads with no masked-out columns.
                    off = max(0, 128 * kt - q0)
                    sc = scps.tile([128, 1024], F32, tag="sc")
                    # high priority so no stale pending fill MM can slot
                    # between the two row-tiled halves (they only run
                    # concurrently when issued back-to-back)
                    with tc.high_priority(offset=1 << 20):
                        nc.tensor.matmul(
                            sc[:, off:512],
                            lhsT=kT[0:64, 128 * kt:128 * (kt + 1)],
                            rhs=qT[0:64, q0 + off:q0 + 512],
                            start=True, stop=True,
                        )
                        nc.tensor.matmul(
                            sc[:, 512:1024 - off],
                            lhsT=kT[64:128, 128 * kt:128 * (kt + 1)],
                            rhs=qT[64:128, q0 + off:q0 + 512],
                            start=True, stop=True,
                        )
                    e = ep.tile([128, 1024], BF16)
                    nc.scalar.activation(
                        e[:, off:1024 - off], sc[:, off:1024 - off],
                        EXPF, scale=SCALE,
                    )
                    if 128 * kt >= q0:  # diagonal block: triangular mask
                        nc.vector.tensor_mul(
                            e[:, off:off + 128], e[:, off:off + 128], tri
                        )
                        nc.vector.tensor_mul(
                            e[:, 512:640], e[:, 512:640], tri,
                        )
                    return e, off

                # one-ktile software pipeline: scores(kt+1) sit between
                # scores(kt) and AV(kt) in the in-order PE stream, so the
                # PE computes them while ACT runs exp(kt) instead of
                # stalling head-of-line at AV(kt). The score pair is
                # emitted BEFORE the fill pops so its two row-tiled
                # matmuls get adjacent scheduler priorities: fills that
                # become ready mid-pair would otherwise cut between the
                # h0/h1 matmuls and serialize them.
                pipe = scores_exp(0)
                for kt in range(nkts):
                    nxt = scores_exp(kt + 1) if kt + 1 < nkts else None
                    # in the last batch, stop draining once the queue is
                    # short: the held-back items give the PE cover during
                    # the final normalize chain at the tail.
                    npop = 0 if (reserve and len(fill) <= 11) else 2
                    for _ in range(npop):
                        if fill:
                            fill.popleft()()
                    e, off = pipe
                    nc.tensor.matmul(
                        y0[:, off:512],
                        lhsT=va0[:, 128 * kt:128 * (kt + 1)],
                        rhs=e[:, off:512],
                        start=(kt == 0), stop=(kt == nkts - 1),
                    )
                    nc.tensor.matmul(
                        y1[:, off:512],
                        lhsT=va1[:, 128 * kt:128 * (kt + 1)],
                        rhs=e[:, 512:1024 - off],
                        start=(kt == 0), stop=(kt == nkts - 1),
                    )
                    pipe = nxt

                # normalize: yn[0:64]=y_h0/den0, yn[64:128]=y_h1/den1.
                # Two K=1 selector matmuls on the PE gather the two
                # denominator rows straight out of the staged ysb tiles
                # and broadcast them across 128 partitions into one PSUM
                # tile (they depend only on the DVE copies, so they never
                # stall the PE); 1/den = exp(-ln(den)) then runs on ACT
                # over the broadcast tile (free-dim bound: same cost as a
                # 2-row tile; ln+exp share the natural_log_exp_and_others
                # table set so there is no table reload), and two DVE
                # tensor_muls apply it. No DMAs anywhere in the chain.
                yn = ynp.tile([128, 512], BF16)
                # the staging copies run at high priority: they release the
                # y0/y1 PSUM banks and gate the selector matmul, and must
                # not queue behind pending proj CASTs in the in-order DVE
                # stream (that convoy stalls the next chunk's score pair).
                ysb0 = nstg.tile([64, 512], F32, tag="ysb0")
                ysb1 = nstg.tile([128, 512], F32, tag="ysb1")
                # bf16 staging of the two denominator rows at their native
                # partitions (the selector matmul needs a bf16 rhs); the
                # rest of the tile is zeroed so the K=65 contraction only
                # picks up the two real rows.
                db = nstg.tile([65, 512], BF16, tag="db")
                with tc.high_priority(offset=1 << 20):
                    nc.vector.tensor_copy(ysb0, y0[0:64, :])
                    nc.vector.tensor_copy(ysb1[64:128, :], y1[64:128, :])
                    nc.vector.memset(db, 0.0)
                    nc.vector.tensor_copy(db[64:65, :], y0[64:65, :])
                    nc.vector.tensor_copy(db[32:33, :], y1[32:33, :])
                rln = nstg.tile([128, 512], F32, tag="rln")
                rrec = nstg.tile([128, 512], BF16, tag="rrec")
                rb = [None]

                def n_bcast():
                    rb[0] = mmps.tile([128, 512], F32, tag="mm",
                                      name="rbps")
                    nc.tensor.matmul(rb[0], lhsT=bsel[0:65, :],
                                     rhs=db, start=True, stop=True)

                def n_act():
                    # high priority: jumps ahead of queued ktile exps on
                    # ACT so the mmps bank (rb) frees sooner and yn's
                    # chain shortens; the one-ktile exp pipe absorbs the
                    # displaced exp's delay.
                    with tc.high_priority(offset=1 << 20):
                        nc.scalar.activation(rln, rb[0], LNF)
                        nc.scalar.activation(rrec, rln, EXPF, scale=-1.0)

                def n_muls():
                    with tc.high_priority(offset=1 << 20):
                        nc.vector.tensor_mul(
                            yn[0:64, :], ysb0[0:64, :], rrec[0:64, :]
                        )
                        nc.vector.tensor_mul(
                            yn[64:128, :], ysb1[64:128, :], rrec[64:128, :]
                        )

                fill.append(n_bcast)
                fill.append(n_act)
                fill.append(n_muls)
                return yn

            def emit_proj_oc(b, qc, yn, oc, spread_out=False):
                t0, q0 = b * tb, 512 * qc
                pp = mmps.tile([128, 512], F32, tag="mm")
                nc.tensor.matmul(
                    pp,
                    lhsT=wp_sb[:, 128 * oc:128 * (oc + 1)],
                    rhs=yn,
                    start=True, stop=True,
                )
                ost = ostg.tile([128, 512], BF16)
                nc.vector.tensor_copy(ost, pp)
                # last batch: gpsimd has no x loads left, so alternate the
                # output stores between sync and gpsimd to halve the
                # serialized store tail.
                eng = nc.gpsimd if (spread_out and oc % 2 == 1) else nc.sync
                eng.dma_start(
                    outp[128 * oc:128 * (oc + 1), t0 + q0:t0 + q0 + 512],
                    ost,
                )

            # ---------- pipelined schedule: a queue of deferred PE work
            # fragments (next batch's QKV chain thirds / v_aug groups /
            # previous chunk's projection) drained two items per attention
            # k-tile, so the in-order PE stream has gap-filling matmuls
            # between ACT-paced iterations.
            from collections import deque

            ngr = 3 * (tb // 512)
            fill = deque()
            xts_c = emit_xt(0, spread=True)
            emit_late_consts()
            tiles_c = alloc_qkv()
            for g in range(ngr):
                emit_qkv_group(xts_c, tiles_c, g)
            va_c = emit_vaug(tiles_c[2])
            for b in range(nb):
                qT, kT, _ = tiles_c
                va0, va1 = va_c
                holders = {}
                tail_steps = []
                if b < nb - 1:
                    xts_n = emit_xt(b + 1)
                    tiles_n = alloc_qkv()
                    for g in range(8):
                        fill.extend(qkv_group_steps(xts_n, tiles_n, g))
                    for g in range(8, ngr):
                        tail_steps.extend(qkv_group_steps(xts_n, tiles_n, g))
                    tail_steps.extend(vaug_steps(tiles_n[2], holders))
                last = b == nb - 1
                hold_proj = []
                for qc in range(nqc):
                    yn = emit_attention_qc(
                        qc, qT, kT, va0, va1, fill,
                        reserve=last,
                    )
                    projs = [
                        (lambda yn=yn, qc=qc, oc=oc: emit_proj_oc(
                            b, qc, yn, oc, spread_out=last))
                        for oc in range(C // 128)
                    ]
                    if last and qc == nqc - 2:
                        # held back: becomes PE cover for the final
                        # normalize chain at the tail drain
                        hold_proj = projs
                        projs = []
                    if qc == nqc - 1:
                        # next batch's QKV tail + vaug (and the held-back
                        # projection) drain ahead of this chunk's proj,
                        # giving the PE dense work while the normalize
                        # chain resolves
                        fill.extend(tail_steps)
                        fill.extend(hold_proj)
                    fill.extend(projs)
                while fill:
                    fill.popleft()()
                if b < nb - 1:
                    xts_c, tiles_c, va_c = xts_n, tiles_n, holders["va"]
    if ELIDE_INCS:
        _elide_unwaited_incs(nc)
    _split_excess_waits(nc)
    return nc


def host_prep(x, w_attn, w_proj, nb=B, tb=T):
    """Slice/cast/transpose inputs per core. Returns in_maps for SPMD."""
    ntok = nb * tb
    x = np.asarray(x, dtype=np.float32).reshape(ntok, C)
    w_attn = np.asarray(w_attn, dtype=np.float32)
    w_proj = np.asarray(w_proj, dtype=np.float32)
    xt = np.ascontiguousarray(x.T).astype(ml_dtypes.bfloat16)
    tri_m = np.triu(np.ones((128, 128), np.float32)).astype(ml_dtypes.bfloat16)
    ident_m = np.eye(128, dtype=np.float32).astype(ml_dtypes.bfloat16)
    bsel_m = np.zeros((128, 128), np.float32)
    bsel_m[64, 0:64] = 1.0
    bsel_m[32, 64:128] = 1.0
    bsel_m = bsel_m.astype(ml_dtypes.bfloat16)
    in_maps = []
    for s in range(NCORES):
        r0 = DC * s
        wq = w_attn[r0:r0 + DC, :]
        wk = w_attn[C + r0:C + r0 + DC, :]
        wv = w_attn[2 * C + r0:2 * C + r0 + DC, :]
        wqkv_t = np.ascontiguousarray(
            np.concatenate([wq, wk, wv], axis=0).T
        ).astype(ml_dtypes.bfloat16)                       # [C, 384]
        wp_t = np.ascontiguousarray(w_proj[:, r0:r0 + DC].T).astype(
            ml_dtypes.bfloat16
        )                                                  # [128, C]
        in_maps.append({
            "xt": xt, "wqkv": wqkv_t, "wp": wp_t,
            "tri": tri_m, "ident": ident_m, "bsel": bsel_m,
        })
    return in_maps


def kernel(x, w_attn, w_proj):
    _patch_tile_drain()
    in_maps = host_prep(x, w_attn, w_proj)
    nc = build_attention_nc()
    res = run_bass_kernel_spmd(nc, in_maps, list(range(NCORES)))
    acc = res.results[0]["outp"].astype(np.float32)
    for r in res.results[1:]:
        acc += r["outp"].astype(np.float32)
    return np.ascontiguousarray(acc.T).reshape(B, T, C).astype(np.float32)

